# revision 13
# baseline (speedup 1.0000x reference)
"""Trainium2 Bass kernel for the OPU (optical matmul + ADC quantize) module.

Math (per r-block of 16 contraction rows, j = k mod 16):
    x_c = x + vmap_lut[j, x+8]
    w_c = w + wmap_lut[j, w+8]
    mm_r = x_c[r] @ w_c[r]                   ([BS,16] @ [16,N])
    adc_r = round(mm_r/16) * 16              (clip never triggers:
                                              |mm| <= 16*8.3^2 < 2048)
    out = sum_r adc_r

Design (validated piecewise on the device):
  - fp16 fused operands: x_c/w_c computed directly in fp16 (integer part
    exact, lut corrections keep ~11 bits; output rel err ~7e-3 << 2e-2).
  - LUT corrections via a runtime-registered custom DVE op (LUT2ACC):
    out = in0 + (in1==c)*lutA + (in1==c+1)*lutB  -> 2 levels per pass,
    8 passes per [128,*] chunk instead of 32 eq/stt passes; in0 chains
    through ping-pong fp16 tiles while in1 stays the raw f32 chunk.
  - quantize+accumulate INSIDE PSUM: 8 accumulation chains (4 token tiles
    x 2 PE row-strips), each seeded with MAGIC = 1.5*2^27 by a K=1 fp16
    matmul (12288 * 16384). f32 ulp at MAGIC is exactly 16, so every
    K=16 r-block matmul accumulated into the chain (start=False) rounds
    its mm_r to a multiple of 16 (RNE) - bit-identical to the reference
    ADC. Zero per-block vector work.
  - PE operands must sit at 32-aligned partition bases and a PSUM chain
    must keep one tile_position: r-block 4a+b reads a copy of the
    corrected tile shifted down by 16b partitions (3 SBUF->SBUF DMAs per
    side) at base 64a, tile_position (64a, 0).
  - final: out[mc] = (acc[mc][0] - 2*MAGIC) + acc[mc][1]; ScalarE does the
    bias-copy of acc0 to SBUF (one PSUM operand per instruction), DVE adds
    acc1, then DMA out - pipelined per token tile as its chains close.
    First/last chunks process W in column halves so the DVE spine starts
    on a smaller first DMA and the trailing matmuls overlap the last chain.

Sharding: 4 token-groups x 2 N-halves (TOK=512, NS=512 per core), which
minimizes per-core correction elements (W half + X quarter = 1M elems).
Host prep: slice + transpose of x to [K, TOK] (layout only, no math).
"""
import numpy as np
from contextlib import ExitStack

import concourse.bass as bass
import concourse.bacc as bacc
import concourse.tile as tile
import concourse.mybir as mybir
from concourse import bass_utils
from concourse import dve_ops
from concourse.dve_spec import Spec, Src0, Src1, C0, C1, C2, Zero, One, select, eq

F32 = mybir.dt.float32
FP16 = mybir.dt.float16

B, S, KDIM, N = 2, 1024, 1024, 1024
BS = B * S
NCORES = 8
TGROUPS, NHALVES = 4, 2          # 4x2 core grid
TOK = BS // TGROUPS              # 512 tokens per core
NS = N // NHALVES                # 512 out-cols per core
KC = KDIM // 128                 # 8 k-chunks of 128 partitions
MC = TOK // 128                  # 4 token tiles of 128
MAGIC = float(3 * 2 ** 26)       # 1.5 * 2^27; f32 ulp there is exactly 16
SEED_A, SEED_B = 12288.0, 16384.0   # fp16 pair with product MAGIC

_cache = {}


def _register_lut2():
    name = "LUT2ACC"
    if name in dve_ops._SUB_OPCODE_FOR_NAME:
        return next(o for o in dve_ops.OPS if o.name == name)
    body = (
        Src0
        + select(eq(Src1, C2), C0, Zero)
        + select(eq(Src1, C2 + One), C1, Zero)
    )

    def ref(in0, in1, c0, c1, c2):
        r = in0.astype(np.float32) + \
            np.where(in1 == c2, c0, 0.0) + np.where(in1 == c2 + 1.0, c1, 0.0)
        return r.astype(np.float32)

    spec = Spec(body=body, reference=ref)
    op = dve_ops.DveOp(name, spec, subdim=False, uops_sha={})
    from concourse.dve_table_gen import dve_ver_for
    from concourse.dve_uop import DveOpSpec
    from concourse.dve_spec import lower, _has_src1
    ver = dve_ver_for("TRN2")
    opcode = max(dve_ops._SUB_OPCODE_FOR_NAME.values()) + 1
    assert opcode < 0x20
    dve_ops._SUB_OPCODE_FOR_NAME[name] = opcode
    lowered = DveOpSpec(name=name, opcode=opcode, uops=lower(spec, ver=ver),
                        rd1_en=_has_src1(spec))
    op.uops_sha[ver] = lowered.sha(ver)
    dve_ops.OPS.append(op)
    dve_ops.CUSTOM_DVE_SPECS[name] = spec
    return op


def _build():
    lut2 = _register_lut2()
    nc = bacc.Bacc("TRN2", target_bir_lowering=False, debug=False,
                   enable_asserts=False, num_devices=NCORES)
    xt_d = nc.dram_tensor("xt", [KDIM, TOK], F32, kind="ExternalInput").ap()
    w_d = nc.dram_tensor("w", [KDIM, NS], F32, kind="ExternalInput").ap()
    lut_d = nc.dram_tensor("luts", [128, 32], F32, kind="ExternalInput").ap()
    out_d = nc.dram_tensor("out", [TOK, NS], F32, kind="ExternalOutput").ap()

    with tile.TileContext(nc) as tc, ExitStack() as ctx:
        const = ctx.enter_context(tc.tile_pool(name="const", bufs=1))
        raw = ctx.enter_context(tc.tile_pool(name="raw", bufs=4))
        cor = ctx.enter_context(tc.tile_pool(name="cor", bufs=3))
        stage = ctx.enter_context(tc.tile_pool(name="stage", bufs=2))
        psum = ctx.enter_context(tc.tile_pool(name="psum", bufs=1, space="PSUM"))

        # --- LUTs arrive host-tiled to 128 partitions and fused into one
        # [128, 32] tensor (vmap cols 0:16, wmap cols 16:32): one DMA.
        luts = const.tile([128, 32], F32, tag="luts")
        nc.sync.dma_start(luts[:], lut_d[:, :])
        
        # --- seed the 8 PSUM chains (mc x strip) with MAGIC
        mg = const.tile([65, 128], FP16, tag="mg")
        on = const.tile([65, NS], FP16, tag="on")
        nc.gpsimd.memset(mg[:], SEED_A)
        nc.gpsimd.memset(on[:], SEED_B)
        accs = []
        for mc in range(MC):
            row = []
            for a in range(2):
                acc = psum.tile([128, NS], F32, tag=f"acc{mc}{a}")
                nc.tensor.matmul(acc[:], mg[64 * a:64 * a + 1, :],
                                 on[64 * a:64 * a + 1, :],
                                 start=True, stop=False,
                                 tile_position=(64 * a, 0))
                row.append(acc)
            accs.append(row)

        def chain(src, lutoff, col0, colw, tag):
            """LUT-correct src[:, col0:col0+colw] -> fp16 tile + 3 shifts."""
            ta = cor.tile([128, colw], FP16, tag=f"ca{tag}")
            tb = cor.tile([128, colw], FP16, tag=f"cb{tag}")
            cur = src[:, col0:col0 + colw]
            for i in range(8):
                dst = (ta, tb)[i % 2]
                nc.vector._custom_dve(
                    lut2, out=dst[:], in0=cur, in1=src[:, col0:col0 + colw],
                    s0=luts[:, lutoff + 2 * i:lutoff + 2 * i + 1],
                    s1=luts[:, lutoff + 2 * i + 1:lutoff + 2 * i + 2],
                    imm2=float(2 * i - 8))
                cur = dst[:]
            res = (ta, tb)[7 % 2]
            sh = [res]
            for b in range(1, 4):
                st = cor.tile([128 - 16 * b, colw], FP16, tag=f"cs{tag}{b}")
                nc.sync.dma_start(st[:], res[16 * b:128, :])
                sh.append(st)
            return sh

        tred = [None] * MC

        def mm_block(wt_sh, xt_sh, col0, colw, kc, mc_major=False, close=False):
            """32 accumulating matmuls for one chunk / one W column range."""
            loops = ([(mc, b, a) for mc in range(MC) for b in range(4)
                      for a in range(2)] if mc_major else
                     [(mc, b, a) for b in range(4) for a in range(2)
                      for mc in range(MC)])
            for i, (mc, b, a) in enumerate(loops):
                p0 = 64 * a
                nc.tensor.matmul(
                    accs[mc][a][:, col0:col0 + colw],
                    xt_sh[b][p0:p0 + 16, 128 * mc:128 * (mc + 1)],
                    wt_sh[b][p0:p0 + 16, :],
                    start=False, stop=close and b == 3,
                    tile_position=(p0, 0))
                if mc_major and close and b == 3 and a == 0:
                    tmc = stage.tile([128, NS], F32, tag="t")
                    tred[mc] = tmc
                    nc.scalar.activation(tred[mc][:], accs[mc][0][:],
                                         mybir.ActivationFunctionType.Copy,
                                         bias=-2.0 * MAGIC)
                if mc_major and close and b == 3 and a == 1:
                    o = stage.tile([128, NS], F32, tag="o")
                    nc.vector.tensor_tensor(o[:], tred[mc][:], accs[mc][1][:],
                                            op=mybir.AluOpType.add)
                    nc.sync.dma_start(out_d[128 * mc:128 * (mc + 1), :], o[:])

        # --- per k-chunk: LUT-correct, shift-copy, accumulating matmuls.
        # First chunk: W in column halves so the DVE spine starts on a
        # smaller DMA. Last chunk: X first, then W halves, so the trailing
        # matmuls and per-mc reduces overlap the final chains.
        HALF = NS // 2
        for kc in range(KC):
            first, last = kc == 0, kc == KC - 1
            if first:
                wrawA = raw.tile([128, HALF], F32, tag="wrawA")
                nc.sync.dma_start(wrawA[:], w_d[0:128, 0:HALF])
                wrawB = raw.tile([128, HALF], F32, tag="wrawB")
                nc.sync.dma_start(wrawB[:], w_d[0:128, HALF:])
            else:
                wraw = raw.tile([128, NS], F32, tag="wraw")
                nc.sync.dma_start(wraw[:], w_d[128 * kc:128 * (kc + 1), :])
            xraw = raw.tile([128, TOK], F32, tag="xraw")
            nc.sync.dma_start(xraw[:], xt_d[128 * kc:128 * (kc + 1), :])

            if first:
                w0 = chain(wrawA, 16, 0, HALF, "w0")
                w1 = chain(wrawB, 16, 0, HALF, "w1")
                xs = chain(xraw, 0, 0, TOK, "x")
                mm_block(w0, xs, 0, HALF, kc)
                mm_block(w1, xs, HALF, HALF, kc)
            elif last:
                xs = chain(xraw, 0, 0, TOK, "x")
                w0 = chain(wraw, 16, 0, HALF, "w0")
                mm_block(w0, xs, 0, HALF, kc)
                w1 = chain(wraw, 16, HALF, HALF, "w1")
                mm_block(w1, xs, HALF, HALF, kc, mc_major=True, close=True)
            else:
                ws = chain(wraw, 16, 0, NS, "w")
                xs = chain(xraw, 0, 0, TOK, "x")
                mm_block(ws, xs, 0, NS, kc)

    nc.compile()
    return nc


def kernel(input, weight, vmap_lut, wmap_lut):
    if "nc" not in _cache:
        _cache["nc"] = _build()
    nc = _cache["nc"]
    x2 = np.asarray(input, dtype=np.float32).reshape(BS, KDIM)
    w = np.asarray(weight, dtype=np.float32)
    vl = np.ascontiguousarray(np.asarray(vmap_lut, dtype=np.float32))
    wl = np.ascontiguousarray(np.asarray(wmap_lut, dtype=np.float32))
    in_maps = []
    for c in range(NCORES):
        tg, nh = divmod(c, NHALVES)
        in_maps.append({
            "xt": np.ascontiguousarray(x2[TOK * tg:TOK * (tg + 1), :].T),
            "w": np.ascontiguousarray(w[:, NS * nh:NS * (nh + 1)]),
            "luts": np.ascontiguousarray(
                np.concatenate([np.tile(vl, (8, 1)), np.tile(wl, (8, 1))],
                               axis=1)),
        })
    res = bass_utils.run_bass_kernel_spmd(nc, in_maps, core_ids=list(range(NCORES)))
    out = np.empty((BS, N), dtype=np.float32)
    for c in range(NCORES):
        tg, nh = divmod(c, NHALVES)
        out[TOK * tg:TOK * (tg + 1), NS * nh:NS * (nh + 1)] = res.results[c]["out"]
    return out.reshape(B, S, N)


# revision 14
# speedup vs baseline: 1.0132x; 1.0132x over previous
"""Trainium2 Bass kernel for the OPU (optical matmul + ADC quantize) module.

Math (per r-block of 16 contraction rows, j = k mod 16):
    x_c = x + vmap_lut[j, x+8]
    w_c = w + wmap_lut[j, w+8]
    mm_r = x_c[r] @ w_c[r]                   ([BS,16] @ [16,N])
    adc_r = round(mm_r/16) * 16              (clip never triggers:
                                              |mm| <= 16*8.3^2 < 2048)
    out = sum_r adc_r

Design (validated piecewise on the device):
  - fp16 fused operands: x_c/w_c computed directly in fp16 (integer part
    exact, lut corrections keep ~11 bits; output rel err ~7e-3 << 2e-2).
  - LUT corrections via a runtime-registered custom DVE op (LUT2ACC):
    out = in0 + (in1==c)*lutA + (in1==c+1)*lutB  -> 2 levels per pass,
    8 passes per [128,*] chunk instead of 32 eq/stt passes; in0 chains
    through ping-pong fp16 tiles while in1 stays the raw f32 chunk.
  - quantize+accumulate INSIDE PSUM: 8 accumulation chains (4 token tiles
    x 2 PE row-strips), each seeded with MAGIC = 1.5*2^27 by a K=1 fp16
    matmul (12288 * 16384). f32 ulp at MAGIC is exactly 16, so every
    K=16 r-block matmul accumulated into the chain (start=False) rounds
    its mm_r to a multiple of 16 (RNE) - bit-identical to the reference
    ADC. Zero per-block vector work.
  - PE operands must sit at 32-aligned partition bases and a PSUM chain
    must keep one tile_position: r-block 4a+b reads a copy of the
    corrected tile shifted down by 16b partitions (3 SBUF->SBUF DMAs per
    side) at base 64a, tile_position (64a, 0).
  - final: out[mc] = (acc[mc][0] - 2*MAGIC) + acc[mc][1]; ScalarE does the
    bias-copy of acc0 to SBUF (one PSUM operand per instruction), DVE adds
    acc1, then DMA out - pipelined per token tile as its chains close.
    First/last chunks process W in column halves so the DVE spine starts
    on a smaller first DMA and the trailing matmuls overlap the last chain.

Sharding: 4 token-groups x 2 N-halves (TOK=512, NS=512 per core), which
minimizes per-core correction elements (W half + X quarter = 1M elems).
Host prep: slice + transpose of x to [K, TOK] (layout only, no math).
"""
import numpy as np
from contextlib import ExitStack

import concourse.bass as bass
import concourse.bacc as bacc
import concourse.tile as tile
import concourse.mybir as mybir
from concourse import bass_utils
from concourse import dve_ops
from concourse.dve_spec import Spec, Src0, Src1, C0, C1, C2, Zero, One, select, eq

F32 = mybir.dt.float32
FP16 = mybir.dt.float16

B, S, KDIM, N = 2, 1024, 1024, 1024
BS = B * S
NCORES = 8
TGROUPS, NHALVES = 4, 2          # 4x2 core grid
TOK = BS // TGROUPS              # 512 tokens per core
NS = N // NHALVES                # 512 out-cols per core
KC = KDIM // 128                 # 8 k-chunks of 128 partitions
MC = TOK // 128                  # 4 token tiles of 128
MAGIC = float(3 * 2 ** 26)       # 1.5 * 2^27; f32 ulp there is exactly 16
SEED_A, SEED_B = 12288.0, 16384.0   # fp16 pair with product MAGIC

_cache = {}


def _register_lut2():
    name = "LUT2ACC"
    if name in dve_ops._SUB_OPCODE_FOR_NAME:
        return next(o for o in dve_ops.OPS if o.name == name)
    body = (
        Src0
        + select(eq(Src1, C2), C0, Zero)
        + select(eq(Src1, C2 + One), C1, Zero)
    )

    def ref(in0, in1, c0, c1, c2):
        r = in0.astype(np.float32) + \
            np.where(in1 == c2, c0, 0.0) + np.where(in1 == c2 + 1.0, c1, 0.0)
        return r.astype(np.float32)

    spec = Spec(body=body, reference=ref)
    op = dve_ops.DveOp(name, spec, subdim=False, uops_sha={})
    from concourse.dve_table_gen import dve_ver_for
    from concourse.dve_uop import DveOpSpec
    from concourse.dve_spec import lower, _has_src1
    ver = dve_ver_for("TRN2")
    opcode = max(dve_ops._SUB_OPCODE_FOR_NAME.values()) + 1
    assert opcode < 0x20
    dve_ops._SUB_OPCODE_FOR_NAME[name] = opcode
    lowered = DveOpSpec(name=name, opcode=opcode, uops=lower(spec, ver=ver),
                        rd1_en=_has_src1(spec))
    op.uops_sha[ver] = lowered.sha(ver)
    dve_ops.OPS.append(op)
    dve_ops.CUSTOM_DVE_SPECS[name] = spec
    return op


def _build():
    lut2 = _register_lut2()
    nc = bacc.Bacc("TRN2", target_bir_lowering=False, debug=False,
                   enable_asserts=False, num_devices=NCORES)
    xt_d = nc.dram_tensor("xt", [KDIM, TOK], F32, kind="ExternalInput").ap()
    w_d = nc.dram_tensor("w", [KDIM, NS], F32, kind="ExternalInput").ap()
    lut_d = nc.dram_tensor("luts", [128, 32], F32, kind="ExternalInput").ap()
    out_d = nc.dram_tensor("out", [TOK, NS], F32, kind="ExternalOutput").ap()

    with tile.TileContext(nc) as tc, ExitStack() as ctx:
        const = ctx.enter_context(tc.tile_pool(name="const", bufs=1))
        raw = ctx.enter_context(tc.tile_pool(name="raw", bufs=4))
        cor = ctx.enter_context(tc.tile_pool(name="cor", bufs=3))
        stage = ctx.enter_context(tc.tile_pool(name="stage", bufs=2))
        psum = ctx.enter_context(tc.tile_pool(name="psum", bufs=1, space="PSUM"))

        # --- LUTs arrive host-tiled to 128 partitions and fused into one
        # [128, 32] tensor (vmap cols 0:16, wmap cols 16:32): one DMA.
        luts = const.tile([128, 32], F32, tag="luts")
        nc.sync.dma_start(luts[:], lut_d[:, :])
        
        # --- seed the 8 PSUM chains (mc x strip) with MAGIC
        mg = const.tile([65, 128], FP16, tag="mg")
        on = const.tile([65, NS], FP16, tag="on")
        nc.gpsimd.memset(mg[:], SEED_A)
        nc.gpsimd.memset(on[:], SEED_B)
        accs = []
        for mc in range(MC):
            row = []
            for a in range(2):
                acc = psum.tile([128, NS], F32, tag=f"acc{mc}{a}")
                nc.tensor.matmul(acc[:], mg[64 * a:64 * a + 1, :],
                                 on[64 * a:64 * a + 1, :],
                                 start=True, stop=False,
                                 tile_position=(64 * a, 0))
                row.append(acc)
            accs.append(row)

        def chain(src, lutoff, col0, colw, tag):
            """LUT-correct src[:, col0:col0+colw] -> fp16 tile + 3 shifts."""
            ta = cor.tile([128, colw], FP16, tag=f"ca{tag}")
            tb = cor.tile([128, colw], FP16, tag=f"cb{tag}")
            cur = src[:, col0:col0 + colw]
            for i in range(8):
                dst = (ta, tb)[i % 2]
                nc.vector._custom_dve(
                    lut2, out=dst[:], in0=cur, in1=src[:, col0:col0 + colw],
                    s0=luts[:, lutoff + 2 * i:lutoff + 2 * i + 1],
                    s1=luts[:, lutoff + 2 * i + 1:lutoff + 2 * i + 2],
                    imm2=float(2 * i - 8))
                cur = dst[:]
            res = (ta, tb)[7 % 2]
            sh = [res]
            for b in range(1, 4):
                st = cor.tile([128 - 16 * b, colw], FP16, tag=f"cs{tag}{b}")
                nc.sync.dma_start(st[:], res[16 * b:128, :])
                sh.append(st)
            return sh

        tred = [None] * MC

        def mm_block(wt_sh, xt_sh, col0, colw, kc, mc_major=False, close=False):
            """32 accumulating matmuls for one chunk / one W column range."""
            loops = ([(mc, b, a) for mc in range(MC) for b in range(4)
                      for a in range(2)] if mc_major else
                     [(mc, b, a) for b in range(4) for a in range(2)
                      for mc in range(MC)])
            for i, (mc, b, a) in enumerate(loops):
                p0 = 64 * a
                nc.tensor.matmul(
                    accs[mc][a][:, col0:col0 + colw],
                    xt_sh[b][p0:p0 + 16, 128 * mc:128 * (mc + 1)],
                    wt_sh[b][p0:p0 + 16, :],
                    start=False, stop=close and b == 3,
                    tile_position=(p0, 0))
                if mc_major and close and b == 3 and a == 0:
                    tmc = stage.tile([128, NS], F32, tag="t")
                    tred[mc] = tmc
                    nc.scalar.activation(tred[mc][:], accs[mc][0][:],
                                         mybir.ActivationFunctionType.Copy,
                                         bias=-2.0 * MAGIC)
                if mc_major and close and b == 3 and a == 1:
                    o = stage.tile([128, NS], F32, tag="o")
                    nc.vector.tensor_tensor(o[:], tred[mc][:], accs[mc][1][:],
                                            op=mybir.AluOpType.add)
                    nc.sync.dma_start(out_d[128 * mc:128 * (mc + 1), :], o[:])

        # --- per k-chunk: LUT-correct, shift-copy, accumulating matmuls.
        # First chunk: W in column halves so the DVE spine starts on a
        # smaller DMA. Last chunk: X first, then W halves, so the trailing
        # matmuls and per-mc reduces overlap the final chains.
        HALF = NS // 2
        for kc in range(KC):
            first, last = kc == 0, kc == KC - 1
            if first:
                wrawA = raw.tile([128, HALF], F32, tag="wrawA")
                nc.sync.dma_start(wrawA[:], w_d[0:128, 0:HALF])
                wrawB = raw.tile([128, HALF], F32, tag="wrawB")
                nc.sync.dma_start(wrawB[:], w_d[0:128, HALF:])
            else:
                wraw = raw.tile([128, NS], F32, tag="wraw")
                nc.sync.dma_start(wraw[:], w_d[128 * kc:128 * (kc + 1), :])
            xraw = raw.tile([128, TOK], F32, tag="xraw")
            nc.sync.dma_start(xraw[:], xt_d[128 * kc:128 * (kc + 1), :])

            if first:
                w0 = chain(wrawA, 16, 0, HALF, "w0")
                w1 = chain(wrawB, 16, 0, HALF, "w1")
                xs = chain(xraw, 0, 0, TOK, "x")
                mm_block(w0, xs, 0, HALF, kc)
                mm_block(w1, xs, HALF, HALF, kc)
            elif last:
                xs = chain(xraw, 0, 0, TOK, "x")
                ws = chain(wraw, 16, 0, NS, "w")
                mm_block(ws, xs, 0, NS, kc, mc_major=True, close=True)
            else:
                ws = chain(wraw, 16, 0, NS, "w")
                xs = chain(xraw, 0, 0, TOK, "x")
                mm_block(ws, xs, 0, NS, kc)

    nc.compile()
    return nc


def kernel(input, weight, vmap_lut, wmap_lut):
    if "nc" not in _cache:
        _cache["nc"] = _build()
    nc = _cache["nc"]
    x2 = np.asarray(input, dtype=np.float32).reshape(BS, KDIM)
    w = np.asarray(weight, dtype=np.float32)
    vl = np.ascontiguousarray(np.asarray(vmap_lut, dtype=np.float32))
    wl = np.ascontiguousarray(np.asarray(wmap_lut, dtype=np.float32))
    in_maps = []
    for c in range(NCORES):
        tg, nh = divmod(c, NHALVES)
        in_maps.append({
            "xt": np.ascontiguousarray(x2[TOK * tg:TOK * (tg + 1), :].T),
            "w": np.ascontiguousarray(w[:, NS * nh:NS * (nh + 1)]),
            "luts": np.ascontiguousarray(
                np.concatenate([np.tile(vl, (8, 1)), np.tile(wl, (8, 1))],
                               axis=1)),
        })
    res = bass_utils.run_bass_kernel_spmd(nc, in_maps, core_ids=list(range(NCORES)))
    out = np.empty((BS, N), dtype=np.float32)
    for c in range(NCORES):
        tg, nh = divmod(c, NHALVES)
        out[TOK * tg:TOK * (tg + 1), NS * nh:NS * (nh + 1)] = res.results[c]["out"]
    return out.reshape(B, S, N)
